# revision 5
# baseline (speedup 1.0000x reference)
"""Dilated local attention (3x3 window, dilation 2) on 8 trn2 NeuronCores.

Problem: B=8, DIM=256, H=W=64, N=4096.
  k_u = unfold(k, 3x3, dil=2, pad=2)            [B, 256, 9, N]   (zero pad)
  attn = softmax(einsum(bdn,bdkn->bkn)/16, k)   [B, 9, N]
  out  = einsum(bkn,bdkn->bdn)                  [B, 256, N]

Sharding: pure data parallel, one batch element per core.

Per-core layout (fp16 on chip):
  channels on partitions (2 chunks of 128), pixels along the free dim.
  k/v are zero-padded per image row to 68x68 on the host, so all 9
  dilated window shifts become pure free-dim AP offsets with reference
  zero-padding semantics preserved (scores at padded taps are exactly 0,
  matching the reference's softmax over zero-padded logits).

Pipeline per core:
  phase 1: DVE products q*k_shift (fp16) -> PE one-hot-column ones-matmuls
           reduce over channels into PSUM scores [9, 4096] (fp32, scale
           1/16 folded into the ones weights).
  softmax: ACT Exp -> e; PE ones-reduce -> den; ACT Ln; ACT Exp(-ln) ->
           recip; PE broadcast to 9 rows; ACT copy to SBUF; DVE mult ->
           attn [9, 4096] fp16.  (No max-subtraction: |logits| <= ~7.)
  phase 2: per offset, broadcast attn row across 128 partitions, DVE
           multiply with shifted v and accumulate (fp16).
"""

import numpy as np

B, DIM, H, W = 8, 256, 64, 64
N = H * W
KS, DIL, PAD = 3, 2, 2
HP, WP = H + 2 * PAD, W + 2 * PAD  # 68, 68
NP = HP * WP  # 4624
NCHUNK = 2  # channel chunks of 128
P = 128
NCORES = 8

_CACHE = {}


def _build_program():
    import concourse.bacc as bacc
    import concourse.tile as tile
    import concourse.mybir as mybir

    f16 = mybir.dt.float16
    f32 = mybir.dt.float32
    MULT = mybir.AluOpType.mult
    ADD = mybir.AluOpType.add
    AF = mybir.ActivationFunctionType

    nc = bacc.Bacc("TRN2", target_bir_lowering=False, debug=False)

    q_d = nc.dram_tensor("q", [P, NCHUNK, N], f16, kind="ExternalInput").ap()
    kp_d = nc.dram_tensor("kp", [P, NCHUNK, NP], f16, kind="ExternalInput").ap()
    vp_d = nc.dram_tensor("vp", [P, NCHUNK, NP], f16, kind="ExternalInput").ap()
    # one-hot column weights (value 1/16) for the per-offset channel
    # reduction: oh[:, 9k:9k+9] has column k equal to 1/16.
    oh_d = nc.dram_tensor("oh", [P, 81], f16, kind="ExternalInput").ap()
    ones9_d = nc.dram_tensor("ones9", [9, 1], f16, kind="ExternalInput").ap()
    ones19_d = nc.dram_tensor("ones19", [1, 9], f16, kind="ExternalInput").ap()
    out_d = nc.dram_tensor("out", [P, NCHUNK, N], f16, kind="ExternalOutput").ap()

    # window offsets in padded row-major coords, row-major (di, dj) order to
    # match torch unfold ordering used by the reference
    offs = [(di * DIL, dj * DIL) for di in range(-1, 2) for dj in range(-1, 2)]

    with tile.TileContext(nc) as tc:
        with (
            tc.tile_pool(name="inp", bufs=1) as inp,
            tc.tile_pool(name="cst", bufs=1) as cst,
            tc.tile_pool(name="sm", bufs=1) as smp,
            tc.tile_pool(name="psum", bufs=1, space="PSUM") as psp,
        ):
            q_sb = inp.tile([P, NCHUNK, N], f16, tag="q")
            kp_sb = inp.tile([P, NCHUNK, NP], f16, tag="kp")
            vp_sb = inp.tile([P, NCHUNK, NP], f16, tag="vp")
            oh_sb = cst.tile([P, 81], f16, tag="oh")
            ones9_sb = cst.tile([9, 1], f16, tag="o9")
            ones19_sb = cst.tile([1, 9], f16, tag="o19")

            for c in range(NCHUNK):
                nc.sync.dma_start(q_sb[:, c], q_d[:, c])
                nc.sync.dma_start(kp_sb[:, c], kp_d[:, c])
                nc.sync.dma_start(vp_sb[:, c], vp_d[:, c])
            nc.sync.dma_start(oh_sb[:, :], oh_d)
            nc.sync.dma_start(ones9_sb[:, :], ones9_d)
            nc.sync.dma_start(ones19_sb[:, :], ones19_d)

            # 4D views: [p, chunk, row, col]
            q_v = q_sb[:, :, :].rearrange("p c (r w) -> p c r w", r=H)
            kp_v = kp_sb[:, :, :].rearrange("p c (r w) -> p c r w", r=HP)
            vp_v = vp_sb[:, :, :].rearrange("p c (r w) -> p c r w", r=HP)

            # ---------------- phase 1: scores ----------------
            s_ps = psp.tile([9, N], f32, tag="ps")
            with tc.tile_pool(name="prod", bufs=3) as prp:
                for k, (di, dj) in enumerate(offs):
                    pr = prp.tile([P, NCHUNK, N], f16, tag="pr")
                    pr_v = pr[:, :, :].rearrange("p c (r w) -> p c r w", r=H)
                    nc.vector.tensor_tensor(
                        pr_v,
                        q_v,
                        kp_v[:, :, PAD + di : PAD + di + H, PAD + dj : PAD + dj + W],
                        MULT,
                    )
                    lhsT = oh_sb[:, 9 * k : 9 * k + 9]
                    for c in range(NCHUNK):
                        for b in range(8):
                            nc.tensor.matmul(
                                s_ps[:, 512 * b : 512 * (b + 1)],
                                lhsT,
                                pr[:, c, 512 * b : 512 * (b + 1)],
                                start=(k == 0 and c == 0),
                                stop=(k == 8 and c == 1),
                            )

            # ---------------- softmax over the 9 taps ----------------
            e_sb = smp.tile([9, N], f16, tag="e")
            nc.scalar.activation(e_sb[:, :], s_ps[:, :], AF.Exp)

            den_ps = psp.tile([1, N], f32, tag="ps")
            for b in range(8):
                nc.tensor.matmul(
                    den_ps[:, 512 * b : 512 * (b + 1)],
                    ones9_sb[:, :],
                    e_sb[:, 512 * b : 512 * (b + 1)],
                    start=True,
                    stop=True,
                )
            ln_sb = smp.tile([1, N], f32, tag="ln")
            nc.scalar.activation(ln_sb[:, :], den_ps[:, :], AF.Ln)
            rec_sb = smp.tile([1, N], f16, tag="rec")
            nc.scalar.activation(rec_sb[:, :], ln_sb[:, :], AF.Exp, scale=-1.0)

            rb_ps = psp.tile([9, N], f32, tag="ps")
            for b in range(8):
                nc.tensor.matmul(
                    rb_ps[:, 512 * b : 512 * (b + 1)],
                    ones19_sb[:, :],
                    rec_sb[:, 512 * b : 512 * (b + 1)],
                    start=True,
                    stop=True,
                )
            rb_sb = smp.tile([9, N], f16, tag="rbs")
            nc.scalar.activation(rb_sb[:, :], rb_ps[:, :], AF.Copy)
            attn = smp.tile([9, N], f16, tag="attn")
            nc.vector.tensor_tensor(attn[:, :], e_sb[:, :], rb_sb[:, :], MULT)

            # ---------------- phase 2: weighted sum of shifted v ----------------
            with (
                tc.tile_pool(name="bc", bufs=2) as bcp,
                tc.tile_pool(name="p2", bufs=3) as p2p,
                tc.tile_pool(name="ac", bufs=2) as acp,
            ):
                for c in range(NCHUNK):
                    acc = None
                    for k, (di, dj) in enumerate(offs):
                        # engines can't address partition base k directly; DMA
                        # the attn row to partition 0, then gpsimd-broadcast
                        row = bcp.tile([1, N], f16, tag="row")
                        nc.sync.dma_start(row[:, :], attn[k : k + 1, :])
                        bc = bcp.tile([P, N], f16, tag="bc")
                        nc.gpsimd.partition_broadcast(bc[:, :], row[:, :])
                        bc_v = bc[:, :].rearrange("p (r w) -> p r w", r=H)
                        p2 = p2p.tile([P, N], f16, tag="p2")
                        p2_v = p2[:, :].rearrange("p (r w) -> p r w", r=H)
                        nc.vector.tensor_tensor(
                            p2_v,
                            vp_v[:, c, PAD + di : PAD + di + H, PAD + dj : PAD + dj + W],
                            bc_v,
                            MULT,
                        )
                        if acc is None:
                            acc = p2
                        else:
                            nacc = acp.tile([P, N], f16, tag="ac")
                            nc.vector.tensor_tensor(nacc[:, :], acc[:, :], p2[:, :], ADD)
                            acc = nacc
                    nc.sync.dma_start(out_d[:, c], acc[:, :])

    nc.compile()
    return nc


def _host_inputs(q, k, v):
    """q,k,v: [B, DIM, N] float32 -> list of per-core input dicts."""
    qh = q.astype(np.float16).reshape(B, NCHUNK, P, N).transpose(0, 2, 1, 3)
    ki = k.astype(np.float16).reshape(B, DIM, H, W)
    vi = v.astype(np.float16).reshape(B, DIM, H, W)
    kp = np.zeros((B, DIM, HP, WP), np.float16)
    vp = np.zeros((B, DIM, HP, WP), np.float16)
    kp[:, :, PAD : PAD + H, PAD : PAD + W] = ki
    vp[:, :, PAD : PAD + H, PAD : PAD + W] = vi
    kp = kp.reshape(B, NCHUNK, P, NP).transpose(0, 2, 1, 3)
    vp = vp.reshape(B, NCHUNK, P, NP).transpose(0, 2, 1, 3)

    oh = np.zeros((P, 81), np.float16)
    for k9 in range(9):
        oh[:, 9 * k9 + k9] = 1.0 / 16.0
    ones9 = np.ones((9, 1), np.float16)
    ones19 = np.ones((1, 9), np.float16)

    ins = []
    for b in range(B):
        ins.append(
            {
                "q": np.ascontiguousarray(qh[b]),
                "kp": np.ascontiguousarray(kp[b]),
                "vp": np.ascontiguousarray(vp[b]),
                "oh": oh,
                "ones9": ones9,
                "ones19": ones19,
            }
        )
    return ins


def kernel(q, k, v, h=H, w=W, _trace=False):
    from concourse.bass_utils import run_bass_kernel_spmd

    q = np.asarray(q, np.float32)
    k = np.asarray(k, np.float32)
    v = np.asarray(v, np.float32)

    if "nc" not in _CACHE:
        _CACHE["nc"] = _build_program()
    nc = _CACHE["nc"]

    ins = _host_inputs(q, k, v)
    res = run_bass_kernel_spmd(nc, ins, core_ids=list(range(NCORES)), trace=_trace)

    outs = []
    for b in range(B):
        o = res.results[b]["out"]  # [128, 2, 4096] fp16
        outs.append(o.transpose(1, 0, 2).reshape(DIM, N))
    full = np.stack(outs).astype(np.float32)
    if _trace:
        return full, res
    return full


# revision 30
# speedup vs baseline: 737.3783x; 737.3783x over previous
"""Dilated local attention (3x3 window, dilation 2) on 8 trn2 NeuronCores.

Problem: B=8, DIM=256, H=W=64, N=4096.
  k_u = unfold(k, 3x3, dil=2, pad=2)            [B, 256, 9, N]   (zero pad)
  attn = softmax(einsum(bdn,bdkn->bkn)/16, k)   [B, 9, N]
  out  = einsum(bkn,bdkn->bdn)                  [B, 256, N]

Sharding: pure data parallel, one batch element per core.

Per-core layout (fp16 on chip):
  channels on partitions (2 chunks of 128), pixels along the free dim.
  k/v are zero-padded per image row to 68x68 on the host, so all 9
  dilated window shifts become pure free-dim AP offsets with reference
  zero-padding semantics preserved (scores at padded taps are exactly 0,
  matching the reference's softmax over zero-padded logits).

The image is processed in two row-halves pipelined against each other
(phase 2 of half 0 is interleaved offset-by-offset with phase 1 of half
1) so softmax chains and PE backlogs hide under the other half's DVE
work.  Per half:
  phase 1: DVE products q*k_shift (fp16) -> PE one-hot-column matmuls
           reduce over channels into PSUM scores [9, 2048] (fp32, scale
           1/16 folded into the one-hot weights).
  softmax: ACT Exp -> e; PE ones-reduce -> den; ACT Ln; ACT Exp(-ln) ->
           recip; PE broadcast to 9 rows; ACT copy; DVE mult -> attn
           (fp16).  No max-subtraction needed: |logits| <= ~7.
  phase 2: per offset, DMA the attn row to partition 0, PE-broadcast it
           across partitions (ones outer product, double-buffered PSUM),
           ACT-evacuate to fp16, DVE multiply with shifted v; the nine
           products are combined by a gpsimd in-place add chain (early
           offsets) plus a short DVE add tree (late offsets), so the
           kernel tail stays short.
"""

import numpy as np

B, DIM, H, W = 8, 256, 64, 64
N = H * W
KS, DIL, PAD = 3, 2, 2
HP, WP = H + 2 * PAD, W + 2 * PAD  # 68, 68
NP = HP * WP  # 4624
NCHUNK = 2  # channel chunks of 128
P = 128
NCORES = 8
HH = H // 2  # rows per half
NH = HH * W  # pixels per half (2048)

_CACHE = {}


def _build_program():
    import concourse.bacc as bacc
    import concourse.tile as tile
    import concourse.mybir as mybir

    f16 = mybir.dt.float16
    f32 = mybir.dt.float32
    MULT = mybir.AluOpType.mult
    ADD = mybir.AluOpType.add
    AF = mybir.ActivationFunctionType

    nc = bacc.Bacc("TRN2", target_bir_lowering=False, debug=False)

    q_d = nc.dram_tensor("q", [P, NCHUNK, N], f16, kind="ExternalInput").ap()
    kp_d = nc.dram_tensor("kp", [P, NCHUNK, NP], f16, kind="ExternalInput").ap()
    vp_d = nc.dram_tensor("vp", [P, NCHUNK, NP], f16, kind="ExternalInput").ap()
    # one-hot column weights (value 1/16) for the per-offset channel
    # reduction: oh[:, 9k:9k+9] has column k equal to 1/16.
    oh_d = nc.dram_tensor("oh", [P, 81], f16, kind="ExternalInput").ap()
    ones9_d = nc.dram_tensor("ones9", [9, 1], f16, kind="ExternalInput").ap()
    ones19_d = nc.dram_tensor("ones19", [1, 9], f16, kind="ExternalInput").ap()
    ones1p_d = nc.dram_tensor("ones1p", [1, P], f16, kind="ExternalInput").ap()
    out_d = nc.dram_tensor("out", [P, NCHUNK, N], f16, kind="ExternalOutput").ap()

    # window offsets, row-major (di, dj) to match torch unfold ordering
    offs = [(di * DIL, dj * DIL) for di in range(-1, 2) for dj in range(-1, 2)]
    NBLK = NH // 512  # 512-wide PSUM blocks per half (4)

    with tile.TileContext(nc) as tc:
        with (
            tc.tile_pool(name="inp", bufs=1) as inp,
            tc.tile_pool(name="kpp", bufs=1) as kpp,
            tc.tile_pool(name="cst", bufs=1) as cst,
            tc.tile_pool(name="sm", bufs=1) as smp,
            tc.tile_pool(name="prod", bufs=3) as prp,
            tc.tile_pool(name="bc", bufs=3) as bcp,
            tc.tile_pool(name="p2", bufs=8) as p2p,
            tc.tile_pool(name="psum", bufs=1, space="PSUM") as psp,
        ):
            q_sb = inp.tile([P, NCHUNK, N], f16, tag="q")
            vp_sb = inp.tile([P, NCHUNK, NP], f16, tag="vp")
            kp_sb = kpp.tile([P, NCHUNK, NP], f16, tag="kp")
            oh_sb = cst.tile([P, 81], f16, tag="oh")
            ones9_sb = cst.tile([9, 1], f16, tag="o9")
            ones19_sb = cst.tile([1, 9], f16, tag="o19")
            ones1p_sb = cst.tile([1, P], f16, tag="o1p")

            nc.sync.dma_start(oh_sb[:, :], oh_d)
            nc.sync.dma_start(ones9_sb[:, :], ones9_d)
            nc.sync.dma_start(ones19_sb[:, :], ones19_d)
            nc.sync.dma_start(ones1p_sb[:, :], ones1p_d)
            # split input loads by (chunk, half), ordered so the first
            # products' dependencies (q+kp of half 0) land first;
            # kp/vp halves overlap by the 4 halo rows
            for h in range(2):
                lo, hi = h * HH * WP, ((h + 1) * HH + 2 * PAD) * WP
                mid, midp = (2 * h + 1) * NH // 2, (lo + hi) // 2
                for c in range(NCHUNK):
                    nc.sync.dma_start(
                        q_sb[:, c, h * NH : mid], q_d[:, c, h * NH : mid]
                    )
                    nc.sync.dma_start(
                        q_sb[:, c, mid : (h + 1) * NH], q_d[:, c, mid : (h + 1) * NH]
                    )
                    nc.sync.dma_start(kp_sb[:, c, lo:midp], kp_d[:, c, lo:midp])
                    nc.sync.dma_start(kp_sb[:, c, midp:hi], kp_d[:, c, midp:hi])
            for h in range(2):
                lo, hi = h * HH * WP, ((h + 1) * HH + 2 * PAD) * WP
                for c in range(NCHUNK):
                    nc.sync.dma_start(vp_sb[:, c, lo:hi], vp_d[:, c, lo:hi])

            # 4D views: [p, chunk, row, col]
            q_v = q_sb[:, :, :].rearrange("p c (r w) -> p c r w", r=H)
            kp_v = kp_sb[:, :, :].rearrange("p c (r w) -> p c r w", r=HP)
            vp_v = vp_sb[:, :, :].rearrange("p c (r w) -> p c r w", r=HP)

            def p1_step(h, s_ps, k, chunk_split=False):
                di, dj = offs[k]
                r0 = h * HH
                pr = prp.tile([P, NCHUNK, NH], f16, tag="pr")
                pr_v = pr[:, :, :].rearrange("p c (r w) -> p c r w", r=HH)
                # chunk-split products only need one chunk's q/kp loaded,
                # so the first ones start after ~2 MB of input DMA
                csplit = (
                    [(c, c + 1) for c in range(NCHUNK)]
                    if chunk_split
                    else [(0, NCHUNK)]
                )
                for c0, c1 in csplit:
                    nc.vector.tensor_tensor(
                        pr_v[:, c0:c1],
                        q_v[:, c0:c1, r0 : r0 + HH, :],
                        kp_v[
                            :,
                            c0:c1,
                            PAD + di + r0 : PAD + di + r0 + HH,
                            PAD + dj : PAD + dj + W,
                        ],
                        MULT,
                    )
                lhsT = oh_sb[:, 9 * k : 9 * k + 9]
                for c in range(NCHUNK):
                    for b in range(NBLK):
                        nc.tensor.matmul(
                            s_ps[:, 512 * b : 512 * (b + 1)],
                            lhsT,
                            pr[:, c, 512 * b : 512 * (b + 1)],
                            start=(k == 0 and c == 0),
                            stop=(k == 8 and c == 1),
                        )

            def softmax(h, s_ps):
                e_sb = smp.tile([9, NH], f16, tag=f"e{h}")
                nc.scalar.activation(e_sb[:, :], s_ps[:, :], AF.Exp)
                den_ps = psp.tile([1, NH], f32, tag=f"s{h}")
                for b in range(NBLK):
                    nc.tensor.matmul(
                        den_ps[:, 512 * b : 512 * (b + 1)],
                        ones9_sb[:, :],
                        e_sb[:, 512 * b : 512 * (b + 1)],
                        start=True,
                        stop=True,
                    )
                ln_sb = smp.tile([1, NH], f32, tag="ln")
                nc.scalar.activation(ln_sb[:, :], den_ps[:, :], AF.Ln)
                rec_sb = smp.tile([1, NH], f16, tag="rec")
                nc.scalar.activation(rec_sb[:, :], ln_sb[:, :], AF.Exp, scale=-1.0)
                rb_ps = psp.tile([9, NH], f32, tag=f"s{h}")
                for b in range(NBLK):
                    nc.tensor.matmul(
                        rb_ps[:, 512 * b : 512 * (b + 1)],
                        ones19_sb[:, :],
                        rec_sb[:, 512 * b : 512 * (b + 1)],
                        start=True,
                        stop=True,
                    )
                rb_sb = smp.tile([9, NH], f16, tag="rb")
                nc.scalar.activation(rb_sb[:, :], rb_ps[:, :], AF.Copy)
                attn = smp.tile([9, NH], f16, tag=f"at{h}")
                nc.vector.tensor_tensor(attn[:, :], e_sb[:, :], rb_sb[:, :], MULT)
                return attn

            def p2_step(h, attn, k, prods, bc_tags):
                di, dj = offs[k]
                r0 = h * HH
                # engines can't address partition base k; DMA the attn row to
                # partition 0, then broadcast it across partitions on PE
                # (ones outer product) and evacuate to SBUF fp16 via ACT
                row = bcp.tile([1, NH], f16, tag="row")
                nc.sync.dma_start(row[:, :], attn[k : k + 1, :])
                bc_ps = psp.tile([P, NH], f32, tag=bc_tags[k % len(bc_tags)])
                for b in range(NBLK):
                    nc.tensor.matmul(
                        bc_ps[:, 512 * b : 512 * (b + 1)],
                        ones1p_sb[:, :],
                        row[:, 512 * b : 512 * (b + 1)],
                        start=True,
                        stop=True,
                    )
                bc = bcp.tile([P, NH], f16, tag="bc")
                nc.scalar.activation(bc[:, :], bc_ps[:, :], AF.Copy)
                bc_v = bc[:, :].rearrange("p (r w) -> p r w", r=HH)
                p2 = p2p.tile([P, NCHUNK, NH], f16, tag="p2")
                for c in range(NCHUNK):
                    p2_v = p2[:, c, :].rearrange("p (r w) -> p r w", r=HH)
                    nc.vector.tensor_tensor(
                        p2_v,
                        vp_v[
                            :,
                            c,
                            PAD + di + r0 : PAD + di + r0 + HH,
                            PAD + dj : PAD + dj + W,
                        ],
                        bc_v,
                        MULT,
                    )
                prods[k] = p2
                # opportunistic early accumulation on gpsimd (in-place chain)
                if k == 1:
                    nc.gpsimd.tensor_tensor(
                        prods[0][:, :, :], prods[0][:, :, :], prods[1][:, :, :],
                        ADD,
                    )
                elif k == 3:
                    nc.gpsimd.tensor_tensor(
                        prods[2][:, :, :], prods[2][:, :, :], prods[3][:, :, :],
                        ADD,
                    )
                elif k == 4 and h == 0:
                    nc.gpsimd.tensor_tensor(
                        prods[0][:, :, :], prods[0][:, :, :], prods[2][:, :, :],
                        ADD,
                    )

            def p2_finish(h, prods):
                # combine the remaining products on DVE; tail stays short
                t1 = p2p.tile([P, NCHUNK, NH], f16, tag="p2")
                nc.vector.tensor_tensor(
                    t1[:, :, :], prods[4][:, :, :], prods[5][:, :, :], ADD
                )
                t2 = p2p.tile([P, NCHUNK, NH], f16, tag="p2")
                nc.vector.tensor_tensor(
                    t2[:, :, :], prods[6][:, :, :], prods[7][:, :, :], ADD
                )
                t3 = p2p.tile([P, NCHUNK, NH], f16, tag="p2")
                nc.vector.tensor_tensor(t3[:, :, :], t1[:, :, :], t2[:, :, :], ADD)
                t4 = p2p.tile([P, NCHUNK, NH], f16, tag="p2")
                nc.vector.tensor_tensor(
                    t4[:, :, :], t3[:, :, :], prods[8][:, :, :], ADD
                )
                t5 = p2p.tile([P, NCHUNK, NH], f16, tag="p2")
                nc.vector.tensor_tensor(
                    t5[:, :, :], t4[:, :, :], prods[0][:, :, :], ADD
                )
                last = t5
                if h == 1:
                    t6 = p2p.tile([P, NCHUNK, NH], f16, tag="p2")
                    nc.vector.tensor_tensor(
                        t6[:, :, :], t5[:, :, :], prods[2][:, :, :], ADD
                    )
                    last = t6
                for c in range(NCHUNK):
                    nc.sync.dma_start(
                        out_d[:, c, h * NH : (h + 1) * NH], last[:, c, :]
                    )

            # pre-warm the ACT function tables (Exp, Ln) during input DMA so
            # no table load lands mid-pipeline
            warm = smp.tile([1, 8], f32, tag="warm")
            nc.vector.memset(warm[:, :], 1.0)
            nc.scalar.activation(warm[:, :], warm[:, :], AF.Exp)
            nc.scalar.activation(warm[:, :], warm[:, :], AF.Ln)

            s0 = psp.tile([9, NH], f32, tag="s0")
            for k in range(9):
                p1_step(0, s0, k, chunk_split=(k < 3))
            a0 = softmax(0, s0)
            # zipper: interleave phase2(0) (lagged so attn(0) is ready) with
            # phase1(1) so the in-order PE stream serves both halves
            s1 = psp.tile([9, NH], f32, tag="s1")
            prods0 = [None] * 9
            for k in range(9):
                p1_step(1, s1, k)
                if k >= 4:
                    p2_step(0, a0, k - 4, prods0, bc_tags=("s0",))
            a1 = softmax(1, s1)
            for k in range(5, 9):
                p2_step(0, a0, k, prods0, bc_tags=("s0", "s1"))
            p2_finish(0, prods0)
            prods1 = [None] * 9
            for k in range(9):
                p2_step(1, a1, k, prods1, bc_tags=("s0", "s1"))
            p2_finish(1, prods1)

    nc.compile()
    return nc


def _host_inputs(q, k, v):
    """q,k,v: [B, DIM, N] float32 -> list of per-core input dicts."""
    qh = q.astype(np.float16).reshape(B, NCHUNK, P, N).transpose(0, 2, 1, 3)
    ki = k.astype(np.float16).reshape(B, DIM, H, W)
    vi = v.astype(np.float16).reshape(B, DIM, H, W)
    kp = np.zeros((B, DIM, HP, WP), np.float16)
    vp = np.zeros((B, DIM, HP, WP), np.float16)
    kp[:, :, PAD : PAD + H, PAD : PAD + W] = ki
    vp[:, :, PAD : PAD + H, PAD : PAD + W] = vi
    kp = kp.reshape(B, NCHUNK, P, NP).transpose(0, 2, 1, 3)
    vp = vp.reshape(B, NCHUNK, P, NP).transpose(0, 2, 1, 3)

    oh = np.zeros((P, 81), np.float16)
    for k9 in range(9):
        oh[:, 9 * k9 + k9] = 1.0 / 16.0
    ones9 = np.ones((9, 1), np.float16)
    ones19 = np.ones((1, 9), np.float16)
    ones1p = np.ones((1, P), np.float16)

    ins = []
    for b in range(B):
        ins.append(
            {
                "q": np.ascontiguousarray(qh[b]),
                "kp": np.ascontiguousarray(kp[b]),
                "vp": np.ascontiguousarray(vp[b]),
                "oh": oh,
                "ones9": ones9,
                "ones19": ones19,
                "ones1p": ones1p,
            }
        )
    return ins


def kernel(q, k, v, h=H, w=W, _trace=False):
    from concourse.bass_utils import run_bass_kernel_spmd

    q = np.asarray(q, np.float32)
    k = np.asarray(k, np.float32)
    v = np.asarray(v, np.float32)

    if "nc" not in _CACHE:
        _CACHE["nc"] = _build_program()
    nc = _CACHE["nc"]

    ins = _host_inputs(q, k, v)
    res = run_bass_kernel_spmd(nc, ins, core_ids=list(range(NCORES)), trace=_trace)

    outs = []
    for b in range(B):
        o = res.results[b]["out"]  # [128, 2, 4096] fp16
        outs.append(o.transpose(1, 0, 2).reshape(DIM, N))
    full = np.stack(outs).astype(np.float32)
    if _trace:
        return full, res
    return full
